# revision 16
# baseline (speedup 1.0000x reference)
"""Trainium2 Bass kernel for a 2-layer GCN + global mean pool + MLP head.

Device work per core (SPMD, shared NEFF):
  - agg = A_norm @ x for the core's 12.5K-node shard: per-edge dma_gather of
    fp16 x rows (256B elems) + one-hot window matmuls into PSUM.
    * 50 gathers (25 blocks x 2 src chunks), round-robined over 4 SWDGE
      queues so descriptor generation uses all 8 Q7 cores.
    * src chunks of 50K rows via signed int16 indices against a centered
      base (the gather ucode sign-extends and MULUS-accumulates).
    * one-hot tiles precomputed on host (graph metadata only) and DMA'd,
      keeping the vector engine almost idle (it contends with SWDGE).
  - h1e = elu(agg.T @ W1 + b1); elementwise on the scalar engine.
  - partial[G, D] += wT_block.T @ h1e with wT = (B^T A_norm).T from host.
  - Host epilogue: sum partials, @W2, mean, fc1/relu/fc2, log_softmax.
"""
import sys
import numpy as np

sys.path.insert(0, "/opt/trn_rl_repo")


# ---------------------------------------------------------------- config ----
class CFG:
    def __init__(self, N=100000, D=128, G=64, n_cores=8, n_chunk=2, blk=512,
                 w=32):
        self.N, self.D, self.G = N, D, G
        self.N_CORES, self.N_CHUNK, self.BLK, self.W = n_cores, n_chunk, blk, w
        self.SHARD = N // n_cores
        self.CHUNK = N // n_chunk
        self.HALF = self.CHUNK // 2  # centered gather base offset
        assert self.HALF <= 32768
        self.N_BLK = (self.SHARD + blk - 1) // blk
        self.SHARD_PAD = self.N_BLK * blk
        self.N_RUN = self.N_BLK * n_chunk
        self.NT = blk // 128


FULL = CFG()


# ---------------------------------------------------------- preprocessing ----
def pack_run_joint(drels, W, blk):
    """Joint greedy window packing across cores: one shared base list; each
    core fills <=128 of its own (sorted) edges per tile."""
    nc_ = len(drels)
    ptrs = [0] * nc_
    tile_ofs = [np.full(len(d), -1, dtype=np.int64) for d in drels]
    wbases = []
    t = 0
    while True:
        lo = blk
        for c in range(nc_):
            if ptrs[c] < len(drels[c]):
                lo = min(lo, int(drels[c][ptrs[c]]))
        if lo >= blk:
            break
        base = min(lo, blk - W)
        for c in range(nc_):
            a = ptrs[c]
            if a >= len(drels[c]):
                continue
            hi = np.searchsorted(drels[c], base + W, side="left")
            j = min(a + 128, hi)
            tile_ofs[c][a:j] = t
            ptrs[c] = j
        wbases.append(base)
        t += 1
    return tile_ofs, wbases


def _sentinel_guard(gidx, oh, t_end):
    """The gather ucode drops trailing-negative indices; make sure the last
    slot of the (half-)gather ending at tile t_end is >= 0 by swapping a
    non-negative slot of the final tile into position."""
    last = t_end * 128 - 1
    if gidx[last] < 0:
        lt = t_end - 1
        cand = np.nonzero(gidx[lt * 128:(lt + 1) * 128] >= 0)[0]
        assert len(cand) > 0, "all-negative tile"
        sw = lt * 128 + int(cand[0])
        gidx[sw], gidx[last] = gidx[last], gidx[sw]
        tmp = oh[lt, cand[0]].copy()
        oh[lt, cand[0]] = oh[lt, 127]
        oh[lt, 127] = tmp


def preprocess(cfg, edge_index, batch):
    import ml_dtypes
    src = np.asarray(edge_index[0], dtype=np.int64)
    dst = np.asarray(edge_index[1], dtype=np.int64)
    batch = np.asarray(batch, dtype=np.int64)
    N, G, W, BLK = cfg.N, cfg.G, cfg.W, cfg.BLK

    deg = np.bincount(dst, minlength=N).astype(np.float32) + 1.0
    dinv = (1.0 / np.sqrt(deg)).astype(np.float32)

    loops = np.arange(N, dtype=np.int64)
    la = np.concatenate([src, loops])
    lb = np.concatenate([dst, loops])
    wnorm = (dinv[la] * dinv[lb]).astype(np.float32)
    flat = batch[lb] * N + la
    wmat = np.bincount(flat, weights=wnorm.astype(np.float64),
                       minlength=G * N).reshape(G, N).astype(np.float32)

    # self-loops are folded into the PSUM init (dinv^2-scaled x block rows
    # via DMA-transpose); only real edges go through the gather stream.
    # Duplicate (src,dst) pairs are merged with summed weights (aggregation
    # is linear), trimming ~8% of the per-edge gather stream.
    norm_full = (dinv[src] * dinv[dst]).astype(np.float32)
    key = dst * np.int64(N) + src
    uk, inv = np.unique(key, return_inverse=True)
    norm_all = np.bincount(inv, weights=norm_full.astype(np.float64)
                           ).astype(np.float32)
    src_all = (uk % N).astype(np.int64)
    dst_all = (uk // N).astype(np.int64)

    cores = []
    for c in range(cfg.N_CORES):
        m = (dst_all >= c * cfg.SHARD) & (dst_all < (c + 1) * cfg.SHARD)
        s_c = src_all[m]
        dl_c = dst_all[m] - c * cfg.SHARD
        nv_c = norm_all[m]
        blk_id = dl_c // BLK
        chunk = s_c // cfg.CHUNK
        o = np.lexsort((dl_c, chunk, blk_id))
        s_c, dl_c, nv_c = s_c[o], dl_c[o], nv_c[o]
        run_id = blk_id[o] * cfg.N_CHUNK + chunk[o]
        run_starts = np.searchsorted(run_id, np.arange(cfg.N_RUN))
        run_ends = np.searchsorted(run_id, np.arange(cfg.N_RUN) + 1)
        cores.append((s_c, dl_c, nv_c, run_starts, run_ends))

    run_T, run_wb = [], []
    core_gidx = [[] for _ in range(cfg.N_CORES)]
    core_oh = [[] for _ in range(cfg.N_CORES)]
    for r in range(cfg.N_RUN):
        bb = (r // cfg.N_CHUNK) * BLK
        ch = r % cfg.N_CHUNK
        drels = []
        for c in range(cfg.N_CORES):
            s_c, dl_c, nv_c, rs, re = cores[c]
            a, b = int(rs[r]), int(re[r])
            drels.append(dl_c[a:b] - bb)
        tile_ofs, wbases = pack_run_joint(drels, W, BLK)
        T_r = max(len(wbases), 1)
        wbases = wbases or [0]
        S_r = T_r * 128
        wb_arr = np.asarray(wbases, dtype=np.int64)
        for c in range(cfg.N_CORES):
            s_c, dl_c, nv_c, rs, re = cores[c]
            a, b = int(rs[r]), int(re[r])
            tile_of = tile_ofs[c]
            assert b == a or (tile_of >= 0).all()
            gidx = np.zeros(S_r, dtype=np.int16)
            oh = np.zeros((T_r, 128, W), dtype=ml_dtypes.float8_e4m3)
            if b > a:
                loads = np.bincount(tile_of, minlength=T_r)
                cum = np.concatenate([[0], np.cumsum(loads)])[:-1]
                pos = np.arange(b - a) - np.repeat(cum, loads)
                slot = tile_of * 128 + pos
                gidx[slot] = (s_c[a:b] - ch * cfg.CHUNK
                              - cfg.HALF).astype(np.int16)
                wrel = (dl_c[a:b] - bb - wb_arr[tile_of]).astype(np.int64)
                oh[tile_of, pos, wrel] = nv_c[a:b].astype(
                    ml_dtypes.float8_e4m3)
                # the gather is issued as two halves; guard both boundaries
                t_half = (T_r + 1) // 2
                _sentinel_guard(gidx, oh, t_half)
                if T_r > t_half:
                    _sentinel_guard(gidx, oh, T_r)
            gw = np.tile(gidx.reshape(S_r // 16, 16).T, (8, 1))
            core_gidx[c].append(gw)
            core_oh[c].append(oh.transpose(1, 0, 2).reshape(128, T_r * W))
        run_T.append(T_r)
        run_wb.append(wbases)

    per_core = []
    for c in range(cfg.N_CORES):
        wT = np.zeros((cfg.SHARD_PAD, G), dtype=ml_dtypes.float8_e4m3)
        wT[:cfg.SHARD] = wmat[:, c * cfg.SHARD:(c + 1) * cfg.SHARD].T
        per_core.append({
            "gidx": np.ascontiguousarray(np.concatenate(core_gidx[c], axis=1)),
            "oh": np.ascontiguousarray(np.concatenate(core_oh[c], axis=1)),
            "wT": wT,
            "dinv2": (dinv[c * cfg.SHARD:(c + 1) * cfg.SHARD] ** 2
                      ).astype(np.float32),
        })

    cnt = np.bincount(batch, minlength=G).astype(np.float32)
    return per_core, cnt, run_T, run_wb


# ---------------------------------------------------------- bass kernel ----
def build_kernel(cfg, run_T, run_wb):
    from concourse import bacc, bass, tile, mybir
    from concourse.masks import make_identity
    f32 = mybir.dt.float32
    f16 = mybir.dt.float16
    bf16 = mybir.dt.bfloat16
    f8e4 = mybir.dt.float8e4
    f8e3 = mybir.dt.float8e3

    T_total = sum(run_T)
    nc = bacc.Bacc("TRN2", target_bir_lowering=False, debug=False,
                   enable_asserts=False, num_swdge_queues=4,
                   dynamic_dma_scratch_size=40960)
    x_t = nc.dram_tensor("x", [cfg.N, cfg.D], f16, kind="ExternalInput")
    xself_t = nc.dram_tensor("x_selfT", [cfg.D, cfg.SHARD_PAD], f8e3,
                             kind="ExternalInput")
    gidx_t = nc.dram_tensor("gidx", [128, 8 * T_total], mybir.dt.int16,
                            kind="ExternalInput")
    oh_t = nc.dram_tensor("oh", [128, cfg.W * T_total], f8e4,
                          kind="ExternalInput")
    wT_t = nc.dram_tensor("wT", [cfg.SHARD_PAD, cfg.G], f8e4,
                          kind="ExternalInput")
    W1_t = nc.dram_tensor("W1", [cfg.D, cfg.D], bf16, kind="ExternalInput")
    b1_t = nc.dram_tensor("b1", [1, cfg.D], f32, kind="ExternalInput")
    out_t = nc.dram_tensor("partial", [cfg.G, cfg.D], f32,
                           kind="ExternalOutput")

    W, NT, BLK = cfg.W, cfg.NT, cfg.BLK
    add, amax = mybir.AluOpType.add, mybir.AluOpType.max

    with tile.TileContext(nc) as tc:
        with (tc.tile_pool(name="const", bufs=1) as cpool,
              tc.tile_pool(name="gbuf", bufs=8) as gpool,
              tc.tile_pool(name="meta", bufs=8) as mpool,
              tc.tile_pool(name="ohp", bufs=8) as opool,
              tc.tile_pool(name="agg", bufs=2) as apool,
              tc.tile_pool(name="xbtp", bufs=4) as xpool,
              tc.tile_pool(name="eluv", bufs=2) as epool,
              tc.tile_pool(name="wt", bufs=4) as wpool,
              tc.tile_pool(name="psA", bufs=2, space="PSUM") as pApool,
              tc.tile_pool(name="psB", bufs=2, space="PSUM") as pBpool,
              tc.tile_pool(name="psR", bufs=2, space="PSUM") as pRpool,
              tc.tile_pool(name="psC", bufs=1, space="PSUM") as pCpool,
              tc.tile_pool(name="outp", bufs=1) as outpool):

            # rolling metadata prefetch PF runs ahead: every load in the
            # (in-order) Sync HWDGE stream is issued well before its consumer
            # so no wait in that stream ever blocks later loads -- otherwise
            # the whole pipeline marches in lockstep at one block per drain.
            PF = 4
            run_off = np.concatenate([[0], np.cumsum(run_T)]).astype(int)

            def load_meta(rr):
                off = int(run_off[rr])
                T_rr = run_T[rr]
                gxs = mpool.tile([128, 8 * T_rr], mybir.dt.int16)
                nc.sync.dma_start(
                    gxs[:], gidx_t.ap()[:, 8 * off:8 * (off + T_rr)])
                ohs = opool.tile([128, T_rr, W], f8e4)
                nc.sync.dma_start(
                    ohs[:].rearrange("p t w -> p (t w)"),
                    oh_t.ap()[:, W * off:W * (off + T_rr)])
                return gxs, ohs

            pre = {}
            for rr in range(min(PF, cfg.N_RUN)):
                pre[rr] = load_meta(rr)

            ident = cpool.tile([128, 128], f16)
            make_identity(nc, ident[:])
            W1s = cpool.tile([128, cfg.D], bf16)
            nc.sync.dma_start(W1s[:], W1_t.ap())
            b1s = cpool.tile([128, cfg.D], f32)
            nc.sync.dma_start(b1s[:], b1_t.ap().to_broadcast((128, cfg.D)))

            psC = pCpool.tile([cfg.G, cfg.D], f32)

            for b in range(cfg.N_BLK):
                # per-block side streams, hoisted to the block top so their
                # (slack, 4-deep) pool waits sit early in the Sync stream
                xbt = xpool.tile([128, BLK], f8e3)
                nc.sync.dma_start(
                    xbt[:], xself_t.ap()[:, b * BLK:(b + 1) * BLK])
                wts = wpool.tile([128, NT, cfg.G], f8e4)
                nc.sync.dma_start(
                    wts[:],
                    wT_t.ap()[b * BLK:(b + 1) * BLK, :]
                        .rearrange("(t p) g -> p t g", p=128))

                psA = None
                for ch in range(cfg.N_CHUNK):
                    r = b * cfg.N_CHUNK + ch
                    T_r = run_T[r]
                    gxs, ohs = pre.pop(r)
                    if r + PF < cfg.N_RUN:
                        pre[r + PF] = load_meta(r + PF)

                    gb = gpool.tile([128, T_r, cfg.D], f16)
                    base_row = ch * cfg.CHUNK + cfg.HALF
                    # one gather per run: the DMASW sem rotation (8 lanes)
                    # caps in-flight SWDGE DMAs at 8, so fewer/bigger gathers
                    # maximize the descriptor runway the DMA engines can
                    # drain ahead (rings hold ~2 full gathers per queue).
                    S_r = T_r * 128
                    nc.gpsimd.dma_gather(
                        out_ap=gb[:],
                        in_ap=x_t.ap()[base_row:cfg.N, :],
                        idxs_ap=gxs[:],
                        num_idxs=S_r,
                        num_idxs_reg=S_r,
                        elem_size=cfg.D,
                        single_packet=False,
                        queue_num=r % 4,
                    )

                    if psA is None:
                        # init psA with the self-loop term: dinv^2-scaled x
                        # rows of this block, host-pretransposed to feat-major
                        psA = pApool.tile([128, BLK], f32)
                        nc.tensor.matmul(out=psA[:], lhsT=ident[:],
                                         rhs=xbt[:], start=True, stop=False)
                    wbs = run_wb[r]
                    for t in range(T_r):
                        last = (ch == cfg.N_CHUNK - 1 and t == T_r - 1)
                        nc.tensor.matmul(
                            out=psA[:, wbs[t]:wbs[t] + W],
                            lhsT=gb[:, t, :],
                            rhs=ohs[:, t, :],
                            start=False, stop=last,
                        )

                # drain agg (feat-major [D x BLK])
                aggs = apool.tile([128, BLK], bf16)
                nc.vector.tensor_copy(out=aggs[:], in_=psA[:])

                # B: h1 = agg.T @ W1  -> psB [node x feat_out]
                psB = pBpool.tile([128, BLK], f32)
                for nt in range(NT):
                    nc.tensor.matmul(out=psB[:, nt * cfg.D:(nt + 1) * cfg.D],
                                     lhsT=aggs[:, nt * 128:(nt + 1) * 128],
                                     rhs=W1s[:], start=True, stop=True)

                # elu(xb) = relu(xb) - relu(1 - exp(xb)), xb = psB + b1.
                # Every DVE op reads at most ONE SBUF operand (the other is
                # PSUM) -- 2-SBUF-port DVE ops get locked out by concurrent
                # SWDGE descriptor generation (measured 100x slowdown).
                xb = epool.tile([128, NT, cfg.D], f32, tag="xb")
                nc.vector.tensor_tensor(
                    out=xb[:],
                    in0=psB[:].rearrange("p (t d) -> p t d", d=cfg.D),
                    in1=b1s[:].unsqueeze(1).broadcast_to((128, NT, cfg.D)),
                    op=add)
                ex = epool.tile([128, NT * cfg.D], f32, tag="ex")
                nc.scalar.activation(
                    out=ex[:], in_=xb[:].rearrange("p t d -> p (t d)"),
                    func=mybir.ActivationFunctionType.Exp)
                rneg = epool.tile([128, NT * cfg.D], f32, tag="rneg")
                nc.scalar.activation(
                    out=rneg[:], in_=ex[:],
                    func=mybir.ActivationFunctionType.Relu,
                    bias=1.0, scale=-1.0)
                rpos = pRpool.tile([128, NT * cfg.D], f32)
                nc.scalar.activation(
                    out=rpos[:], in_=xb[:].rearrange("p t d -> p (t d)"),
                    func=mybir.ActivationFunctionType.Relu)
                h1e = epool.tile([128, NT * cfg.D], bf16, tag="h1e")
                nc.vector.tensor_tensor(
                    out=h1e[:], in0=rpos[:], in1=rneg[:],
                    op=mybir.AluOpType.subtract)

                # C: partial += wT_block.T @ h1e
                for nt in range(NT):
                    nc.tensor.matmul(
                        out=psC[:],
                        lhsT=wts[:, nt, :],
                        rhs=h1e[:, nt * cfg.D:(nt + 1) * cfg.D],
                        start=(b == 0 and nt == 0),
                        stop=(b == cfg.N_BLK - 1 and nt == NT - 1),
                    )

            outs = outpool.tile([cfg.G, cfg.D], f32)
            nc.vector.tensor_copy(out=outs[:], in_=psC[:])
            nc.sync.dma_start(out_t.ap(), outs[:])

    nc.compile()
    return nc


# ------------------------------------------------------------- epilogue ----
def epilogue(partials, cnt, W2, b2, fc1_W, fc1_b, fc2_W, fc2_b):
    g_sum = np.sum(partials, axis=0, dtype=np.float32)
    S = g_sum @ W2 + cnt[:, None] * b2[None, :]
    mean = S / np.maximum(cnt, 1.0)[:, None]
    z = np.maximum(mean @ fc1_W + fc1_b[None, :], 0.0)
    z = z @ fc2_W + fc2_b[None, :]
    zmax = z.max(axis=1, keepdims=True)
    lse = np.log(np.sum(np.exp(z - zmax), axis=1, keepdims=True)) + zmax
    return (z - lse).astype(np.float32)


_NC_CACHE = {}


def run_on_device(cfg, per_core, run_T, run_wb, x):
    key = (tuple(run_T), tuple(tuple(w) for w in run_wb))
    if key not in _NC_CACHE:
        _NC_CACHE.clear()
        _NC_CACHE[key] = build_kernel(cfg, run_T, run_wb)
    nc = _NC_CACHE[key]
    import ml_dtypes
    xf = np.asarray(x, np.float32)
    xp = np.ascontiguousarray(xf.astype(np.float16))
    in_maps = []
    for c in range(cfg.N_CORES):
        s = per_core[c]
        xs = np.zeros((cfg.SHARD_PAD, cfg.D), dtype=ml_dtypes.float8_e3m4)
        xs[:cfg.SHARD] = (xf[c * cfg.SHARD:(c + 1) * cfg.SHARD]
                          * s["dinv2"][:, None]).astype(ml_dtypes.float8_e3m4)
        in_maps.append({
            "x": xp, "x_selfT": np.ascontiguousarray(xs.T),
            "gidx": s["gidx"], "oh": s["oh"],
            "wT": s["wT"], "W1": None, "b1": None,
        })
    return nc, in_maps


def kernel(x, edge_index, batch, W1, b1, W2, b2, fc1_W, fc1_b, fc2_W, fc2_b):
    from concourse import bass_utils
    cfg = FULL
    per_core, cnt, run_T, run_wb = preprocess(cfg, edge_index, batch)
    nc, in_maps = run_on_device(cfg, per_core, run_T, run_wb, x)
    import ml_dtypes
    W1f = np.ascontiguousarray(
        np.asarray(W1, dtype=np.float32).astype(ml_dtypes.bfloat16))
    b1f = np.asarray(b1, dtype=np.float32).reshape(1, cfg.D)
    for m in in_maps:
        m["W1"] = W1f
        m["b1"] = b1f
    res = bass_utils.run_bass_kernel_spmd(
        nc, in_maps, core_ids=list(range(cfg.N_CORES)))
    partials = [res.results[c]["partial"] for c in range(cfg.N_CORES)]
    out = epilogue(partials, cnt,
                   np.asarray(W2, np.float32), np.asarray(b2, np.float32),
                   np.asarray(fc1_W, np.float32), np.asarray(fc1_b, np.float32),
                   np.asarray(fc2_W, np.float32), np.asarray(fc2_b, np.float32))
    return out



# revision 17
# speedup vs baseline: 1.0144x; 1.0144x over previous
"""Trainium2 Bass kernel for a 2-layer GCN + global mean pool + MLP head.

Device work per core (SPMD, shared NEFF):
  - agg = A_norm @ x for the core's 12.5K-node shard: per-edge dma_gather of
    fp16 x rows (256B elems) + one-hot window matmuls into PSUM.
    * 50 gathers (25 blocks x 2 src chunks), round-robined over 4 SWDGE
      queues so descriptor generation uses all 8 Q7 cores.
    * src chunks of 50K rows via signed int16 indices against a centered
      base (the gather ucode sign-extends and MULUS-accumulates).
    * one-hot tiles precomputed on host (graph metadata only) and DMA'd,
      keeping the vector engine almost idle (it contends with SWDGE).
  - h1e = elu(agg.T @ W1 + b1); elementwise on the scalar engine.
  - partial[G, D] += wT_block.T @ h1e with wT = (B^T A_norm).T from host.
  - Host epilogue: sum partials, @W2, mean, fc1/relu/fc2, log_softmax.
"""
import sys
import numpy as np

sys.path.insert(0, "/opt/trn_rl_repo")


# ---------------------------------------------------------------- config ----
class CFG:
    def __init__(self, N=100000, D=128, G=64, n_cores=8, n_chunk=2, blk=512,
                 w=32):
        self.N, self.D, self.G = N, D, G
        self.N_CORES, self.N_CHUNK, self.BLK, self.W = n_cores, n_chunk, blk, w
        self.SHARD = N // n_cores
        self.CHUNK = N // n_chunk
        self.HALF = self.CHUNK // 2  # centered gather base offset
        assert self.HALF <= 32768
        self.N_BLK = (self.SHARD + blk - 1) // blk
        self.SHARD_PAD = self.N_BLK * blk
        self.N_RUN = self.N_BLK * n_chunk
        self.NT = blk // 128


FULL = CFG()


# ---------------------------------------------------------- preprocessing ----
def pack_run_joint(drels, W, blk):
    """Joint greedy window packing across cores: one shared base list; each
    core fills <=128 of its own (sorted) edges per tile."""
    nc_ = len(drels)
    ptrs = [0] * nc_
    tile_ofs = [np.full(len(d), -1, dtype=np.int64) for d in drels]
    wbases = []
    t = 0
    while True:
        lo = blk
        for c in range(nc_):
            if ptrs[c] < len(drels[c]):
                lo = min(lo, int(drels[c][ptrs[c]]))
        if lo >= blk:
            break
        base = min(lo, blk - W)
        for c in range(nc_):
            a = ptrs[c]
            if a >= len(drels[c]):
                continue
            hi = np.searchsorted(drels[c], base + W, side="left")
            j = min(a + 128, hi)
            tile_ofs[c][a:j] = t
            ptrs[c] = j
        wbases.append(base)
        t += 1
    return tile_ofs, wbases


def _sentinel_guard(gidx, oh, t_end):
    """The gather ucode drops trailing-negative indices; make sure the last
    slot of the (half-)gather ending at tile t_end is >= 0 by swapping a
    non-negative slot of the final tile into position."""
    last = t_end * 128 - 1
    if gidx[last] < 0:
        lt = t_end - 1
        cand = np.nonzero(gidx[lt * 128:(lt + 1) * 128] >= 0)[0]
        assert len(cand) > 0, "all-negative tile"
        sw = lt * 128 + int(cand[0])
        gidx[sw], gidx[last] = gidx[last], gidx[sw]
        tmp = oh[lt, cand[0]].copy()
        oh[lt, cand[0]] = oh[lt, 127]
        oh[lt, 127] = tmp


def preprocess(cfg, edge_index, batch):
    import ml_dtypes
    src = np.asarray(edge_index[0], dtype=np.int64)
    dst = np.asarray(edge_index[1], dtype=np.int64)
    batch = np.asarray(batch, dtype=np.int64)
    N, G, W, BLK = cfg.N, cfg.G, cfg.W, cfg.BLK

    deg = np.bincount(dst, minlength=N).astype(np.float32) + 1.0
    dinv = (1.0 / np.sqrt(deg)).astype(np.float32)

    loops = np.arange(N, dtype=np.int64)
    la = np.concatenate([src, loops])
    lb = np.concatenate([dst, loops])
    wnorm = (dinv[la] * dinv[lb]).astype(np.float32)
    flat = batch[lb] * N + la
    wmat = np.bincount(flat, weights=wnorm.astype(np.float64),
                       minlength=G * N).reshape(G, N).astype(np.float32)

    # self-loops are folded into the PSUM init (dinv^2-scaled x block rows
    # via DMA-transpose); only real edges go through the gather stream.
    # Duplicate (src,dst) pairs are merged with summed weights (aggregation
    # is linear), trimming ~8% of the per-edge gather stream.
    norm_full = (dinv[src] * dinv[dst]).astype(np.float32)
    key = dst * np.int64(N) + src
    uk, inv = np.unique(key, return_inverse=True)
    norm_all = np.bincount(inv, weights=norm_full.astype(np.float64)
                           ).astype(np.float32)
    src_all = (uk % N).astype(np.int64)
    dst_all = (uk // N).astype(np.int64)

    cores = []
    for c in range(cfg.N_CORES):
        m = (dst_all >= c * cfg.SHARD) & (dst_all < (c + 1) * cfg.SHARD)
        s_c = src_all[m]
        dl_c = dst_all[m] - c * cfg.SHARD
        nv_c = norm_all[m]
        blk_id = dl_c // BLK
        chunk = s_c // cfg.CHUNK
        o = np.lexsort((dl_c, chunk, blk_id))
        s_c, dl_c, nv_c = s_c[o], dl_c[o], nv_c[o]
        run_id = blk_id[o] * cfg.N_CHUNK + chunk[o]
        run_starts = np.searchsorted(run_id, np.arange(cfg.N_RUN))
        run_ends = np.searchsorted(run_id, np.arange(cfg.N_RUN) + 1)
        cores.append((s_c, dl_c, nv_c, run_starts, run_ends))

    run_T, run_wb = [], []
    core_gidx = [[] for _ in range(cfg.N_CORES)]
    core_oh = [[] for _ in range(cfg.N_CORES)]
    for r in range(cfg.N_RUN):
        bb = (r // cfg.N_CHUNK) * BLK
        ch = r % cfg.N_CHUNK
        drels = []
        for c in range(cfg.N_CORES):
            s_c, dl_c, nv_c, rs, re = cores[c]
            a, b = int(rs[r]), int(re[r])
            drels.append(dl_c[a:b] - bb)
        tile_ofs, wbases = pack_run_joint(drels, W, BLK)
        T_r = max(len(wbases), 1)
        wbases = wbases or [0]
        S_r = T_r * 128
        wb_arr = np.asarray(wbases, dtype=np.int64)
        for c in range(cfg.N_CORES):
            s_c, dl_c, nv_c, rs, re = cores[c]
            a, b = int(rs[r]), int(re[r])
            tile_of = tile_ofs[c]
            assert b == a or (tile_of >= 0).all()
            gidx = np.zeros(S_r, dtype=np.int16)
            oh = np.zeros((T_r, 128, W), dtype=ml_dtypes.float8_e4m3)
            if b > a:
                loads = np.bincount(tile_of, minlength=T_r)
                cum = np.concatenate([[0], np.cumsum(loads)])[:-1]
                pos = np.arange(b - a) - np.repeat(cum, loads)
                slot = tile_of * 128 + pos
                gidx[slot] = (s_c[a:b] - ch * cfg.CHUNK
                              - cfg.HALF).astype(np.int16)
                wrel = (dl_c[a:b] - bb - wb_arr[tile_of]).astype(np.int64)
                oh[tile_of, pos, wrel] = nv_c[a:b].astype(
                    ml_dtypes.float8_e4m3)
                # the gather is issued as two halves; guard both boundaries
                t_half = (T_r + 1) // 2
                _sentinel_guard(gidx, oh, t_half)
                if T_r > t_half:
                    _sentinel_guard(gidx, oh, T_r)
            gw = np.tile(gidx.reshape(S_r // 16, 16).T, (8, 1))
            core_gidx[c].append(gw)
            core_oh[c].append(oh.transpose(1, 0, 2).reshape(128, T_r * W))
        run_T.append(T_r)
        run_wb.append(wbases)

    per_core = []
    for c in range(cfg.N_CORES):
        wT = np.zeros((cfg.SHARD_PAD, G), dtype=ml_dtypes.float8_e4m3)
        wT[:cfg.SHARD] = wmat[:, c * cfg.SHARD:(c + 1) * cfg.SHARD].T
        per_core.append({
            "gidx": np.ascontiguousarray(np.concatenate(core_gidx[c], axis=1)),
            "oh": np.ascontiguousarray(np.concatenate(core_oh[c], axis=1)),
            "wT": wT,
            "dinv2": (dinv[c * cfg.SHARD:(c + 1) * cfg.SHARD] ** 2
                      ).astype(np.float32),
        })

    cnt = np.bincount(batch, minlength=G).astype(np.float32)
    return per_core, cnt, run_T, run_wb


# ---------------------------------------------------------- bass kernel ----
def build_kernel(cfg, run_T, run_wb):
    from concourse import bacc, bass, tile, mybir
    from concourse.masks import make_identity
    f32 = mybir.dt.float32
    f16 = mybir.dt.float16
    bf16 = mybir.dt.bfloat16
    f8e4 = mybir.dt.float8e4
    f8e3 = mybir.dt.float8e3

    T_total = sum(run_T)
    nc = bacc.Bacc("TRN2", target_bir_lowering=False, debug=False,
                   enable_asserts=False, num_swdge_queues=4,
                   dynamic_dma_scratch_size=65536)
    x_t = nc.dram_tensor("x", [cfg.N, cfg.D], f16, kind="ExternalInput")
    xself_t = nc.dram_tensor("x_selfT", [cfg.D, cfg.SHARD_PAD], f8e3,
                             kind="ExternalInput")
    gidx_t = nc.dram_tensor("gidx", [128, 8 * T_total], mybir.dt.int16,
                            kind="ExternalInput")
    oh_t = nc.dram_tensor("oh", [128, cfg.W * T_total], f8e4,
                          kind="ExternalInput")
    wT_t = nc.dram_tensor("wT", [cfg.SHARD_PAD, cfg.G], f8e4,
                          kind="ExternalInput")
    W1_t = nc.dram_tensor("W1", [cfg.D, cfg.D], bf16, kind="ExternalInput")
    b1_t = nc.dram_tensor("b1", [1, cfg.D], f32, kind="ExternalInput")
    out_t = nc.dram_tensor("partial", [cfg.G, cfg.D], f32,
                           kind="ExternalOutput")

    W, NT, BLK = cfg.W, cfg.NT, cfg.BLK
    add, amax = mybir.AluOpType.add, mybir.AluOpType.max

    with tile.TileContext(nc) as tc:
        with (tc.tile_pool(name="const", bufs=1) as cpool,
              tc.tile_pool(name="gbuf", bufs=8) as gpool,
              tc.tile_pool(name="meta", bufs=8) as mpool,
              tc.tile_pool(name="ohp", bufs=8) as opool,
              tc.tile_pool(name="agg", bufs=2) as apool,
              tc.tile_pool(name="xbtp", bufs=4) as xpool,
              tc.tile_pool(name="eluv", bufs=2) as epool,
              tc.tile_pool(name="wt", bufs=4) as wpool,
              tc.tile_pool(name="psA", bufs=2, space="PSUM") as pApool,
              tc.tile_pool(name="psB", bufs=2, space="PSUM") as pBpool,
              tc.tile_pool(name="psR", bufs=2, space="PSUM") as pRpool,
              tc.tile_pool(name="psC", bufs=1, space="PSUM") as pCpool,
              tc.tile_pool(name="outp", bufs=1) as outpool):

            # rolling metadata prefetch PF runs ahead: every load in the
            # (in-order) Sync HWDGE stream is issued well before its consumer
            # so no wait in that stream ever blocks later loads -- otherwise
            # the whole pipeline marches in lockstep at one block per drain.
            PF = 4
            run_off = np.concatenate([[0], np.cumsum(run_T)]).astype(int)

            def load_meta(rr):
                off = int(run_off[rr])
                T_rr = run_T[rr]
                gxs = mpool.tile([128, 8 * T_rr], mybir.dt.int16)
                nc.sync.dma_start(
                    gxs[:], gidx_t.ap()[:, 8 * off:8 * (off + T_rr)])
                ohs = opool.tile([128, T_rr, W], f8e4)
                nc.sync.dma_start(
                    ohs[:].rearrange("p t w -> p (t w)"),
                    oh_t.ap()[:, W * off:W * (off + T_rr)])
                return gxs, ohs

            pre = {}
            for rr in range(min(PF, cfg.N_RUN)):
                pre[rr] = load_meta(rr)

            ident = cpool.tile([128, 128], f16)
            make_identity(nc, ident[:])
            W1s = cpool.tile([128, cfg.D], bf16)
            nc.sync.dma_start(W1s[:], W1_t.ap())
            b1s = cpool.tile([128, cfg.D], f32)
            nc.sync.dma_start(b1s[:], b1_t.ap().to_broadcast((128, cfg.D)))

            psC = pCpool.tile([cfg.G, cfg.D], f32)

            for b in range(cfg.N_BLK):
                # per-block side streams, hoisted to the block top so their
                # (slack, 4-deep) pool waits sit early in the Sync stream
                xbt = xpool.tile([128, BLK], f8e3)
                nc.sync.dma_start(
                    xbt[:], xself_t.ap()[:, b * BLK:(b + 1) * BLK])
                wts = wpool.tile([128, NT, cfg.G], f8e4)
                nc.sync.dma_start(
                    wts[:],
                    wT_t.ap()[b * BLK:(b + 1) * BLK, :]
                        .rearrange("(t p) g -> p t g", p=128))

                psA = None
                for ch in range(cfg.N_CHUNK):
                    r = b * cfg.N_CHUNK + ch
                    T_r = run_T[r]
                    gxs, ohs = pre.pop(r)
                    if r + PF < cfg.N_RUN:
                        pre[r + PF] = load_meta(r + PF)

                    gb = gpool.tile([128, T_r, cfg.D], f16)
                    base_row = ch * cfg.CHUNK + cfg.HALF
                    # one gather per run: the DMASW sem rotation (8 lanes)
                    # caps in-flight SWDGE DMAs at 8, so fewer/bigger gathers
                    # maximize the descriptor runway the DMA engines can
                    # drain ahead (rings hold ~2 full gathers per queue).
                    S_r = T_r * 128
                    nc.gpsimd.dma_gather(
                        out_ap=gb[:],
                        in_ap=x_t.ap()[base_row:cfg.N, :],
                        idxs_ap=gxs[:],
                        num_idxs=S_r,
                        num_idxs_reg=S_r,
                        elem_size=cfg.D,
                        single_packet=False,
                        queue_num=r % 4,
                    )

                    if psA is None:
                        # init psA with the self-loop term: dinv^2-scaled x
                        # rows of this block, host-pretransposed to feat-major
                        psA = pApool.tile([128, BLK], f32)
                        nc.tensor.matmul(out=psA[:], lhsT=ident[:],
                                         rhs=xbt[:], start=True, stop=False)
                    wbs = run_wb[r]
                    for t in range(T_r):
                        last = (ch == cfg.N_CHUNK - 1 and t == T_r - 1)
                        nc.tensor.matmul(
                            out=psA[:, wbs[t]:wbs[t] + W],
                            lhsT=gb[:, t, :],
                            rhs=ohs[:, t, :],
                            start=False, stop=last,
                        )

                # drain agg (feat-major [D x BLK])
                aggs = apool.tile([128, BLK], bf16)
                nc.vector.tensor_copy(out=aggs[:], in_=psA[:])

                # B: h1 = agg.T @ W1  -> psB [node x feat_out]
                psB = pBpool.tile([128, BLK], f32)
                for nt in range(NT):
                    nc.tensor.matmul(out=psB[:, nt * cfg.D:(nt + 1) * cfg.D],
                                     lhsT=aggs[:, nt * 128:(nt + 1) * 128],
                                     rhs=W1s[:], start=True, stop=True)

                # elu(xb) = relu(xb) - relu(1 - exp(xb)), xb = psB + b1.
                # Every DVE op reads at most ONE SBUF operand (the other is
                # PSUM) -- 2-SBUF-port DVE ops get locked out by concurrent
                # SWDGE descriptor generation (measured 100x slowdown).
                xb = epool.tile([128, NT, cfg.D], f32, tag="xb")
                nc.vector.tensor_tensor(
                    out=xb[:],
                    in0=psB[:].rearrange("p (t d) -> p t d", d=cfg.D),
                    in1=b1s[:].unsqueeze(1).broadcast_to((128, NT, cfg.D)),
                    op=add)
                ex = epool.tile([128, NT * cfg.D], f32, tag="ex")
                nc.scalar.activation(
                    out=ex[:], in_=xb[:].rearrange("p t d -> p (t d)"),
                    func=mybir.ActivationFunctionType.Exp)
                rneg = epool.tile([128, NT * cfg.D], f32, tag="rneg")
                nc.scalar.activation(
                    out=rneg[:], in_=ex[:],
                    func=mybir.ActivationFunctionType.Relu,
                    bias=1.0, scale=-1.0)
                rpos = pRpool.tile([128, NT * cfg.D], f32)
                nc.scalar.activation(
                    out=rpos[:], in_=xb[:].rearrange("p t d -> p (t d)"),
                    func=mybir.ActivationFunctionType.Relu)
                h1e = epool.tile([128, NT * cfg.D], bf16, tag="h1e")
                nc.vector.tensor_tensor(
                    out=h1e[:], in0=rpos[:], in1=rneg[:],
                    op=mybir.AluOpType.subtract)

                # C: partial += wT_block.T @ h1e
                for nt in range(NT):
                    nc.tensor.matmul(
                        out=psC[:],
                        lhsT=wts[:, nt, :],
                        rhs=h1e[:, nt * cfg.D:(nt + 1) * cfg.D],
                        start=(b == 0 and nt == 0),
                        stop=(b == cfg.N_BLK - 1 and nt == NT - 1),
                    )

            outs = outpool.tile([cfg.G, cfg.D], f32)
            nc.vector.tensor_copy(out=outs[:], in_=psC[:])
            nc.sync.dma_start(out_t.ap(), outs[:])

    nc.compile()
    return nc


# ------------------------------------------------------------- epilogue ----
def epilogue(partials, cnt, W2, b2, fc1_W, fc1_b, fc2_W, fc2_b):
    g_sum = np.sum(partials, axis=0, dtype=np.float32)
    S = g_sum @ W2 + cnt[:, None] * b2[None, :]
    mean = S / np.maximum(cnt, 1.0)[:, None]
    z = np.maximum(mean @ fc1_W + fc1_b[None, :], 0.0)
    z = z @ fc2_W + fc2_b[None, :]
    zmax = z.max(axis=1, keepdims=True)
    lse = np.log(np.sum(np.exp(z - zmax), axis=1, keepdims=True)) + zmax
    return (z - lse).astype(np.float32)


_NC_CACHE = {}


def run_on_device(cfg, per_core, run_T, run_wb, x):
    key = (tuple(run_T), tuple(tuple(w) for w in run_wb))
    if key not in _NC_CACHE:
        _NC_CACHE.clear()
        _NC_CACHE[key] = build_kernel(cfg, run_T, run_wb)
    nc = _NC_CACHE[key]
    import ml_dtypes
    xf = np.asarray(x, np.float32)
    xp = np.ascontiguousarray(xf.astype(np.float16))
    in_maps = []
    for c in range(cfg.N_CORES):
        s = per_core[c]
        xs = np.zeros((cfg.SHARD_PAD, cfg.D), dtype=ml_dtypes.float8_e3m4)
        xs[:cfg.SHARD] = (xf[c * cfg.SHARD:(c + 1) * cfg.SHARD]
                          * s["dinv2"][:, None]).astype(ml_dtypes.float8_e3m4)
        in_maps.append({
            "x": xp, "x_selfT": np.ascontiguousarray(xs.T),
            "gidx": s["gidx"], "oh": s["oh"],
            "wT": s["wT"], "W1": None, "b1": None,
        })
    return nc, in_maps


def kernel(x, edge_index, batch, W1, b1, W2, b2, fc1_W, fc1_b, fc2_W, fc2_b):
    from concourse import bass_utils
    cfg = FULL
    per_core, cnt, run_T, run_wb = preprocess(cfg, edge_index, batch)
    nc, in_maps = run_on_device(cfg, per_core, run_T, run_wb, x)
    import ml_dtypes
    W1f = np.ascontiguousarray(
        np.asarray(W1, dtype=np.float32).astype(ml_dtypes.bfloat16))
    b1f = np.asarray(b1, dtype=np.float32).reshape(1, cfg.D)
    for m in in_maps:
        m["W1"] = W1f
        m["b1"] = b1f
    res = bass_utils.run_bass_kernel_spmd(
        nc, in_maps, core_ids=list(range(cfg.N_CORES)))
    partials = [res.results[c]["partial"] for c in range(cfg.N_CORES)]
    out = epilogue(partials, cnt,
                   np.asarray(W2, np.float32), np.asarray(b2, np.float32),
                   np.asarray(fc1_W, np.float32), np.asarray(fc1_b, np.float32),
                   np.asarray(fc2_W, np.float32), np.asarray(fc2_b, np.float32))
    return out



# revision 27
# speedup vs baseline: 1.0757x; 1.0604x over previous
"""Trainium2 Bass kernel for a 2-layer GCN + global mean pool + MLP head.

Device work per core (SPMD, shared NEFF):
  - agg = A_norm @ x for the core's 12.5K-node shard: per-edge dma_gather of
    fp16 x rows (256B elems) + one-hot window matmuls into PSUM.
    * 50 gathers (25 blocks x 2 src chunks), round-robined over 4 SWDGE
      queues so descriptor generation uses all 8 Q7 cores.
    * src chunks of 50K rows via signed int16 indices against a centered
      base (the gather ucode sign-extends and MULUS-accumulates).
    * one-hot tiles precomputed on host (graph metadata only) and DMA'd,
      keeping the vector engine almost idle (it contends with SWDGE).
  - h1e = elu(agg.T @ W1 + b1); elementwise on the scalar engine.
  - partial[G, D] += wT_block.T @ h1e with wT = (B^T A_norm).T from host.
  - Host epilogue: sum partials, @W2, mean, fc1/relu/fc2, log_softmax.
"""
import sys
import numpy as np

sys.path.insert(0, "/opt/trn_rl_repo")


# ---------------------------------------------------------------- config ----
class CFG:
    def __init__(self, N=100000, D=128, G=64, n_cores=8, n_chunk=2, blk=512,
                 w=48, wseg=64):
        self.N, self.D, self.G = N, D, G
        self.N_CORES, self.N_CHUNK, self.BLK, self.W = n_cores, n_chunk, blk, w
        self.WSEG = wseg
        self.SHARD = N // n_cores
        self.CHUNK = N // n_chunk
        self.HALF = self.CHUNK // 2  # centered gather base offset
        assert self.HALF <= 32768
        self.N_BLK = (self.SHARD + blk - 1) // blk
        self.SHARD_PAD = self.N_BLK * blk
        self.N_RUN = self.N_BLK * n_chunk
        self.NT = blk // 128


FULL = CFG()


# ---------------------------------------------------------- preprocessing ----
def pack_run_joint(drels, W, blk):
    """Joint greedy window packing across cores: one shared base list; each
    core fills <=128 of its own (sorted) edges per tile."""
    nc_ = len(drels)
    ptrs = [0] * nc_
    tile_ofs = [np.full(len(d), -1, dtype=np.int64) for d in drels]
    wbases = []
    t = 0
    while True:
        lo = blk
        for c in range(nc_):
            if ptrs[c] < len(drels[c]):
                lo = min(lo, int(drels[c][ptrs[c]]))
        if lo >= blk:
            break
        base = min(lo, blk - W)
        for c in range(nc_):
            a = ptrs[c]
            if a >= len(drels[c]):
                continue
            hi = np.searchsorted(drels[c], base + W, side="left")
            j = min(a + 128, hi)
            tile_ofs[c][a:j] = t
            ptrs[c] = j
        wbases.append(base)
        t += 1
    return tile_ofs, wbases


def _sentinel_guard(gidx, oh, t_end):
    """The gather ucode drops trailing-negative indices; make sure the last
    slot of the (half-)gather ending at tile t_end is >= 0 by swapping a
    non-negative slot of the final tile into position."""
    last = t_end * 128 - 1
    if gidx[last] < 0:
        lt = t_end - 1
        cand = np.nonzero(gidx[lt * 128:(lt + 1) * 128] >= 0)[0]
        assert len(cand) > 0, "all-negative tile"
        sw = lt * 128 + int(cand[0])
        gidx[sw], gidx[last] = gidx[last], gidx[sw]
        tmp = oh[lt, cand[0]].copy()
        oh[lt, cand[0]] = oh[lt, 127]
        oh[lt, 127] = tmp


def preprocess(cfg, edge_index, batch):
    import ml_dtypes
    src = np.asarray(edge_index[0], dtype=np.int64)
    dst = np.asarray(edge_index[1], dtype=np.int64)
    batch = np.asarray(batch, dtype=np.int64)
    N, G, W, BLK = cfg.N, cfg.G, cfg.W, cfg.BLK

    deg = np.bincount(dst, minlength=N).astype(np.float32) + 1.0
    dinv = (1.0 / np.sqrt(deg)).astype(np.float32)

    loops = np.arange(N, dtype=np.int64)
    la = np.concatenate([src, loops])
    lb = np.concatenate([dst, loops])
    wnorm = (dinv[la] * dinv[lb]).astype(np.float32)
    flat = batch[lb] * N + la
    wmat = np.bincount(flat, weights=wnorm.astype(np.float64),
                       minlength=G * N).reshape(G, N).astype(np.float32)

    # self-loops are folded into the PSUM init (dinv^2-scaled x block rows
    # via DMA-transpose); only real edges go through the gather stream.
    # Duplicate (src,dst) pairs are merged with summed weights (aggregation
    # is linear), trimming ~8% of the per-edge gather stream.
    norm_full = (dinv[src] * dinv[dst]).astype(np.float32)
    key = dst * np.int64(N) + src
    uk, inv = np.unique(key, return_inverse=True)
    norm_all = np.bincount(inv, weights=norm_full.astype(np.float64)
                           ).astype(np.float32)
    src_all = (uk % N).astype(np.int64)
    dst_all = (uk // N).astype(np.int64)

    # Split each core's edges into two streams:
    #  - SEG: the first edge of each unique src, per block, window-sorted.
    #    These rows are laid out contiguously (slot order) in a per-core
    #    permuted copy of x, so the whole stream is bulk affine DMA with
    #    zero Q7 descriptor-generation work.
    #  - REP: all remaining edges, delivered via per-edge dma_gather.
    WSEG = cfg.WSEG
    cores = []
    cores_seg = []
    for c in range(cfg.N_CORES):
        m = (dst_all >= c * cfg.SHARD) & (dst_all < (c + 1) * cfg.SHARD)
        s_a = src_all[m]
        dl_a = dst_all[m] - c * cfg.SHARD
        nv_a = norm_all[m]
        blk_a = dl_a // BLK
        wrel_a = dl_a - blk_a * BLK
        o1 = np.lexsort((wrel_a, blk_a))
        s_a, dl_a, nv_a = s_a[o1], dl_a[o1], nv_a[o1]
        blk_a, wrel_a = blk_a[o1], wrel_a[o1]
        _, first_pos = np.unique(s_a, return_index=True)
        isf = np.zeros(len(s_a), dtype=bool)
        isf[first_pos] = True
        # seg stream, already sorted by (block, wrel)
        sb = blk_a[isf]
        seg_starts = np.searchsorted(sb, np.arange(cfg.N_BLK))
        seg_ends = np.searchsorted(sb, np.arange(cfg.N_BLK) + 1)
        cores_seg.append((s_a[isf], wrel_a[isf], nv_a[isf],
                          seg_starts, seg_ends))
        # repeat stream through the existing run machinery
        s_c, dl_c, nv_c = s_a[~isf], dl_a[~isf], nv_a[~isf]
        blk_id = dl_c // BLK
        chunk = s_c // cfg.CHUNK
        o = np.lexsort((dl_c, chunk, blk_id))
        s_c, dl_c, nv_c = s_c[o], dl_c[o], nv_c[o]
        run_id = blk_id[o] * cfg.N_CHUNK + chunk[o]
        run_starts = np.searchsorted(run_id, np.arange(cfg.N_RUN))
        run_ends = np.searchsorted(run_id, np.arange(cfg.N_RUN) + 1)
        cores.append((s_c, dl_c, nv_c, run_starts, run_ends))

    # SEG joint packing per block (shared tile/window structure: one NEFF)
    seg_T, seg_wb = [], []
    core_segsrc = [[] for _ in range(cfg.N_CORES)]
    core_segoh = [[] for _ in range(cfg.N_CORES)]
    for b in range(cfg.N_BLK):
        drels = []
        for c in range(cfg.N_CORES):
            _, wr, _, ss, se = cores_seg[c]
            drels.append(wr[int(ss[b]):int(se[b])])
        tile_ofs, wbases = pack_run_joint(drels, WSEG, BLK)
        T_b = max(len(wbases), 1)
        wbases = wbases or [0]
        wb_arr = np.asarray(wbases, dtype=np.int64)
        for c in range(cfg.N_CORES):
            sr, wr, nv, ss, se = cores_seg[c]
            a, e = int(ss[b]), int(se[b])
            tile_of = tile_ofs[c]
            srcmap = np.full(T_b * 128, -1, dtype=np.int64)
            oh = np.zeros((T_b, 128, WSEG), dtype=ml_dtypes.float8_e4m3)
            if e > a:
                loads = np.bincount(tile_of, minlength=T_b)
                cum = np.concatenate([[0], np.cumsum(loads)])[:-1]
                pos = np.arange(e - a) - np.repeat(cum, loads)
                slot = tile_of * 128 + pos
                srcmap[slot] = sr[a:e]
                wrel = (wr[a:e] - wb_arr[tile_of]).astype(np.int64)
                oh[tile_of, pos, wrel] = nv[a:e].astype(
                    ml_dtypes.float8_e4m3)
            core_segsrc[c].append(srcmap)
            core_segoh[c].append(oh.transpose(1, 0, 2).reshape(128,
                                                               T_b * WSEG))
        seg_T.append(T_b)
        seg_wb.append(wbases)

    run_T, run_wb = [], []
    core_gidx = [[] for _ in range(cfg.N_CORES)]
    core_oh = [[] for _ in range(cfg.N_CORES)]
    for r in range(cfg.N_RUN):
        bb = (r // cfg.N_CHUNK) * BLK
        ch = r % cfg.N_CHUNK
        drels = []
        for c in range(cfg.N_CORES):
            s_c, dl_c, nv_c, rs, re = cores[c]
            a, b = int(rs[r]), int(re[r])
            drels.append(dl_c[a:b] - bb)
        tile_ofs, wbases = pack_run_joint(drels, W, BLK)
        T_r = max(len(wbases), 1)
        wbases = wbases or [0]
        S_r = T_r * 128
        wb_arr = np.asarray(wbases, dtype=np.int64)
        for c in range(cfg.N_CORES):
            s_c, dl_c, nv_c, rs, re = cores[c]
            a, b = int(rs[r]), int(re[r])
            tile_of = tile_ofs[c]
            assert b == a or (tile_of >= 0).all()
            gidx = np.zeros(S_r, dtype=np.int16)
            oh = np.zeros((T_r, 128, W), dtype=ml_dtypes.float8_e4m3)
            if b > a:
                loads = np.bincount(tile_of, minlength=T_r)
                cum = np.concatenate([[0], np.cumsum(loads)])[:-1]
                pos = np.arange(b - a) - np.repeat(cum, loads)
                slot = tile_of * 128 + pos
                gidx[slot] = (s_c[a:b] - ch * cfg.CHUNK
                              - cfg.HALF).astype(np.int16)
                wrel = (dl_c[a:b] - bb - wb_arr[tile_of]).astype(np.int64)
                oh[tile_of, pos, wrel] = nv_c[a:b].astype(
                    ml_dtypes.float8_e4m3)
                # the gather is issued as two halves; guard both boundaries
                t_half = (T_r + 1) // 2
                _sentinel_guard(gidx, oh, t_half)
                if T_r > t_half:
                    _sentinel_guard(gidx, oh, T_r)
            gw = np.tile(gidx.reshape(S_r // 16, 16).T, (8, 1))
            core_gidx[c].append(gw)
            core_oh[c].append(oh.transpose(1, 0, 2).reshape(128, T_r * W))
        run_T.append(T_r)
        run_wb.append(wbases)

    per_core = []
    for c in range(cfg.N_CORES):
        wT = np.zeros((cfg.SHARD_PAD, G), dtype=ml_dtypes.float8_e4m3)
        wT[:cfg.SHARD] = wmat[:, c * cfg.SHARD:(c + 1) * cfg.SHARD].T
        per_core.append({
            "gidx": np.ascontiguousarray(np.concatenate(core_gidx[c], axis=1)),
            "oh": np.ascontiguousarray(np.concatenate(core_oh[c], axis=1)),
            "wT": wT,
            "dinv2": (dinv[c * cfg.SHARD:(c + 1) * cfg.SHARD] ** 2
                      ).astype(np.float32),
            "segsrc": np.concatenate(core_segsrc[c]),
            "ohseg": np.ascontiguousarray(
                np.concatenate(core_segoh[c], axis=1)),
            # shared NEFF metadata (identical for every core)
            "seg_T": seg_T,
            "seg_wb": seg_wb,
        })

    cnt = np.bincount(batch, minlength=G).astype(np.float32)
    return per_core, cnt, run_T, run_wb


# ---------------------------------------------------------- bass kernel ----
def build_kernel(cfg, run_T, run_wb, seg_T, seg_wb):
    from concourse import bacc, bass, tile, mybir
    from concourse.masks import make_identity
    f32 = mybir.dt.float32
    f16 = mybir.dt.float16
    bf16 = mybir.dt.bfloat16
    f8e4 = mybir.dt.float8e4
    f8e3 = mybir.dt.float8e3

    T_total = sum(run_T)
    Tseg_total = sum(seg_T)
    WSEG = cfg.WSEG
    nc = bacc.Bacc("TRN2", target_bir_lowering=False, debug=False,
                   enable_asserts=False, num_swdge_queues=4,
                   dynamic_dma_scratch_size=40960)
    x_t = nc.dram_tensor("x", [cfg.N, cfg.D], f16, kind="ExternalInput")
    xself_t = nc.dram_tensor("x_selfT", [cfg.D, cfg.SHARD_PAD], f8e3,
                             kind="ExternalInput")
    gidx_t = nc.dram_tensor("gidx", [128, 8 * T_total], mybir.dt.int16,
                            kind="ExternalInput")
    oh_t = nc.dram_tensor("oh", [128, cfg.W * T_total], f8e4,
                          kind="ExternalInput")
    xperm_t = nc.dram_tensor("xperm", [128, cfg.D * Tseg_total], f16,
                             kind="ExternalInput")
    ohseg_t = nc.dram_tensor("ohseg", [128, WSEG * Tseg_total], f8e4,
                             kind="ExternalInput")
    wT_t = nc.dram_tensor("wT", [cfg.SHARD_PAD, cfg.G], f8e4,
                          kind="ExternalInput")
    W1_t = nc.dram_tensor("W1", [cfg.D, cfg.D], bf16, kind="ExternalInput")
    b1_t = nc.dram_tensor("b1", [1, cfg.D], f32, kind="ExternalInput")
    out_t = nc.dram_tensor("partial", [cfg.G, cfg.D], f32,
                           kind="ExternalOutput")

    W, NT, BLK = cfg.W, cfg.NT, cfg.BLK
    add, amax = mybir.AluOpType.add, mybir.AluOpType.max

    with tile.TileContext(nc) as tc:
        with (tc.tile_pool(name="const", bufs=1) as cpool,
              tc.tile_pool(name="gbuf", bufs=6) as gpool,
              tc.tile_pool(name="meta", bufs=6) as mpool,
              tc.tile_pool(name="ohp", bufs=6) as opool,
              tc.tile_pool(name="segx", bufs=2) as spool,
              tc.tile_pool(name="sohp", bufs=2) as sopool,
              tc.tile_pool(name="agg", bufs=2) as apool,
              tc.tile_pool(name="xbtp", bufs=4) as xpool,
              tc.tile_pool(name="eluv", bufs=2) as epool,
              tc.tile_pool(name="wt", bufs=4) as wpool,
              tc.tile_pool(name="psA", bufs=2, space="PSUM") as pApool,
              tc.tile_pool(name="psB", bufs=2, space="PSUM") as pBpool,
              tc.tile_pool(name="psR", bufs=2, space="PSUM") as pRpool,
              tc.tile_pool(name="psC", bufs=1, space="PSUM") as pCpool,
              tc.tile_pool(name="outp", bufs=1) as outpool):

            # rolling metadata prefetch PF runs ahead: every load in the
            # (in-order) Sync HWDGE stream is issued well before its consumer
            # so no wait in that stream ever blocks later loads -- otherwise
            # the whole pipeline marches in lockstep at one block per drain.
            PF = 4
            run_off = np.concatenate([[0], np.cumsum(run_T)]).astype(int)

            def load_meta(rr):
                off = int(run_off[rr])
                T_rr = run_T[rr]
                gxs = mpool.tile([128, 8 * T_rr], mybir.dt.int16)
                nc.sync.dma_start(
                    gxs[:], gidx_t.ap()[:, 8 * off:8 * (off + T_rr)])
                ohs = opool.tile([128, T_rr, W], f8e4)
                nc.sync.dma_start(
                    ohs[:].rearrange("p t w -> p (t w)"),
                    oh_t.ap()[:, W * off:W * (off + T_rr)])
                return gxs, ohs

            pre = {}
            for rr in range(min(PF, cfg.N_RUN)):
                pre[rr] = load_meta(rr)

            ident = cpool.tile([128, 128], f16)
            make_identity(nc, ident[:])
            W1s = cpool.tile([128, cfg.D], bf16)
            nc.sync.dma_start(W1s[:], W1_t.ap())
            b1s = cpool.tile([128, cfg.D], f32)
            nc.sync.dma_start(b1s[:], b1_t.ap().to_broadcast((128, cfg.D)))

            psC = pCpool.tile([cfg.G, cfg.D], f32)

            seg_off = np.concatenate([[0], np.cumsum(seg_T)]).astype(int)
            for b in range(cfg.N_BLK):
                # per-block side streams, hoisted to the block top so their
                # (slack, 4-deep) pool waits sit early in the Sync stream
                xbt = xpool.tile([128, BLK], f8e3)
                nc.sync.dma_start(
                    xbt[:], xself_t.ap()[:, b * BLK:(b + 1) * BLK])
                wts = wpool.tile([128, NT, cfg.G], f8e4)
                nc.sync.dma_start(
                    wts[:],
                    wT_t.ap()[b * BLK:(b + 1) * BLK, :]
                        .rearrange("(t p) g -> p t g", p=128))

                # SEG stream: first-appearance rows, pre-permuted on the host
                # into slot order -- one big affine DMA, no Q7 descriptors.
                T_b = seg_T[b]
                so = int(seg_off[b])
                segx = spool.tile([128, T_b, cfg.D], f16)
                nc.sync.dma_start(
                    segx[:].rearrange("p t d -> p (t d)"),
                    xperm_t.ap()[:, cfg.D * so:cfg.D * (so + T_b)])
                sohs = sopool.tile([128, T_b, WSEG], f8e4)
                nc.sync.dma_start(
                    sohs[:].rearrange("p t w -> p (t w)"),
                    ohseg_t.ap()[:, WSEG * so:WSEG * (so + T_b)])

                # init psA with the self-loop term: dinv^2-scaled x rows of
                # this block, host-pretransposed to feat-major
                psA = pApool.tile([128, BLK], f32)
                nc.tensor.matmul(out=psA[:], lhsT=ident[:],
                                 rhs=xbt[:], start=True, stop=False)
                swbs = seg_wb[b]
                for t in range(T_b):
                    nc.tensor.matmul(
                        out=psA[:, swbs[t]:swbs[t] + WSEG],
                        lhsT=segx[:, t, :],
                        rhs=sohs[:, t, :],
                        start=False, stop=False,
                    )

                for ch in range(cfg.N_CHUNK):
                    r = b * cfg.N_CHUNK + ch
                    T_r = run_T[r]
                    gxs, ohs = pre.pop(r)
                    if r + PF < cfg.N_RUN:
                        pre[r + PF] = load_meta(r + PF)

                    gb = gpool.tile([128, T_r, cfg.D], f16)
                    base_row = ch * cfg.CHUNK + cfg.HALF
                    # one gather per run: the DMASW sem rotation (8 lanes)
                    # caps in-flight SWDGE DMAs at 8, so fewer/bigger gathers
                    # maximize the descriptor runway the DMA engines can
                    # drain ahead (rings hold ~2 full gathers per queue).
                    S_r = T_r * 128
                    nc.gpsimd.dma_gather(
                        out_ap=gb[:],
                        in_ap=x_t.ap()[base_row:cfg.N, :],
                        idxs_ap=gxs[:],
                        num_idxs=S_r,
                        num_idxs_reg=S_r,
                        elem_size=cfg.D,
                        single_packet=False,
                        queue_num=r % 4,
                    )

                    wbs = run_wb[r]
                    for t in range(T_r):
                        last = (ch == cfg.N_CHUNK - 1 and t == T_r - 1)
                        nc.tensor.matmul(
                            out=psA[:, wbs[t]:wbs[t] + W],
                            lhsT=gb[:, t, :],
                            rhs=ohs[:, t, :],
                            start=False, stop=last,
                        )

                # drain agg (feat-major [D x BLK])
                aggs = apool.tile([128, BLK], bf16)
                nc.vector.tensor_copy(out=aggs[:], in_=psA[:])

                # B: h1 = agg.T @ W1  -> psB [node x feat_out]
                psB = pBpool.tile([128, BLK], f32)
                for nt in range(NT):
                    nc.tensor.matmul(out=psB[:, nt * cfg.D:(nt + 1) * cfg.D],
                                     lhsT=aggs[:, nt * 128:(nt + 1) * 128],
                                     rhs=W1s[:], start=True, stop=True)

                # elu(xb) = relu(xb) - relu(1 - exp(xb)), xb = psB + b1.
                # Every DVE op reads at most ONE SBUF operand (the other is
                # PSUM) -- 2-SBUF-port DVE ops get locked out by concurrent
                # SWDGE descriptor generation (measured 100x slowdown).
                xb = epool.tile([128, NT, cfg.D], f32, tag="xb")
                nc.vector.tensor_tensor(
                    out=xb[:],
                    in0=psB[:].rearrange("p (t d) -> p t d", d=cfg.D),
                    in1=b1s[:].unsqueeze(1).broadcast_to((128, NT, cfg.D)),
                    op=add)
                ex = epool.tile([128, NT * cfg.D], f32, tag="ex")
                nc.scalar.activation(
                    out=ex[:], in_=xb[:].rearrange("p t d -> p (t d)"),
                    func=mybir.ActivationFunctionType.Exp)
                rneg = epool.tile([128, NT * cfg.D], f32, tag="rneg")
                nc.scalar.activation(
                    out=rneg[:], in_=ex[:],
                    func=mybir.ActivationFunctionType.Relu,
                    bias=1.0, scale=-1.0)
                rpos = pRpool.tile([128, NT * cfg.D], f32)
                nc.scalar.activation(
                    out=rpos[:], in_=xb[:].rearrange("p t d -> p (t d)"),
                    func=mybir.ActivationFunctionType.Relu)
                h1e = epool.tile([128, NT * cfg.D], bf16, tag="h1e")
                nc.vector.tensor_tensor(
                    out=h1e[:], in0=rpos[:], in1=rneg[:],
                    op=mybir.AluOpType.subtract)

                # C: partial += wT_block.T @ h1e
                for nt in range(NT):
                    nc.tensor.matmul(
                        out=psC[:],
                        lhsT=wts[:, nt, :],
                        rhs=h1e[:, nt * cfg.D:(nt + 1) * cfg.D],
                        start=(b == 0 and nt == 0),
                        stop=(b == cfg.N_BLK - 1 and nt == NT - 1),
                    )

            outs = outpool.tile([cfg.G, cfg.D], f32)
            nc.vector.tensor_copy(out=outs[:], in_=psC[:])
            nc.sync.dma_start(out_t.ap(), outs[:])

    nc.compile()
    return nc


# ------------------------------------------------------------- epilogue ----
def epilogue(partials, cnt, W2, b2, fc1_W, fc1_b, fc2_W, fc2_b):
    g_sum = np.sum(partials, axis=0, dtype=np.float32)
    S = g_sum @ W2 + cnt[:, None] * b2[None, :]
    mean = S / np.maximum(cnt, 1.0)[:, None]
    z = np.maximum(mean @ fc1_W + fc1_b[None, :], 0.0)
    z = z @ fc2_W + fc2_b[None, :]
    zmax = z.max(axis=1, keepdims=True)
    lse = np.log(np.sum(np.exp(z - zmax), axis=1, keepdims=True)) + zmax
    return (z - lse).astype(np.float32)


_NC_CACHE = {}


def run_on_device(cfg, per_core, run_T, run_wb, x):
    seg_T = per_core[0]["seg_T"]
    seg_wb = per_core[0]["seg_wb"]
    key = (tuple(run_T), tuple(tuple(w) for w in run_wb),
           tuple(seg_T), tuple(tuple(w) for w in seg_wb))
    if key not in _NC_CACHE:
        _NC_CACHE.clear()
        _NC_CACHE[key] = build_kernel(cfg, run_T, run_wb, seg_T, seg_wb)
    nc = _NC_CACHE[key]
    import ml_dtypes
    xf = np.asarray(x, np.float32)
    xp = np.ascontiguousarray(xf.astype(np.float16))
    Tseg_total = sum(seg_T)
    in_maps = []
    for c in range(cfg.N_CORES):
        s = per_core[c]
        xs = np.zeros((cfg.SHARD_PAD, cfg.D), dtype=ml_dtypes.float8_e3m4)
        xs[:cfg.SHARD] = (xf[c * cfg.SHARD:(c + 1) * cfg.SHARD]
                          * s["dinv2"][:, None]).astype(ml_dtypes.float8_e3m4)
        # per-core node-relabeled copy of x: seg slot order, SBUF layout
        segsrc = s["segsrc"]
        rows = np.zeros((Tseg_total * 128, cfg.D), dtype=np.float16)
        valid = segsrc >= 0
        rows[valid] = xp[segsrc[valid]]
        xperm = np.ascontiguousarray(
            rows.reshape(Tseg_total, 128, cfg.D)
                .transpose(1, 0, 2).reshape(128, Tseg_total * cfg.D))
        in_maps.append({
            "x": xp, "x_selfT": np.ascontiguousarray(xs.T),
            "gidx": s["gidx"], "oh": s["oh"],
            "xperm": xperm, "ohseg": s["ohseg"],
            "wT": s["wT"], "W1": None, "b1": None,
        })
    return nc, in_maps


def kernel(x, edge_index, batch, W1, b1, W2, b2, fc1_W, fc1_b, fc2_W, fc2_b):
    from concourse import bass_utils
    cfg = FULL
    per_core, cnt, run_T, run_wb = preprocess(cfg, edge_index, batch)
    nc, in_maps = run_on_device(cfg, per_core, run_T, run_wb, x)
    import ml_dtypes
    W1f = np.ascontiguousarray(
        np.asarray(W1, dtype=np.float32).astype(ml_dtypes.bfloat16))
    b1f = np.asarray(b1, dtype=np.float32).reshape(1, cfg.D)
    for m in in_maps:
        m["W1"] = W1f
        m["b1"] = b1f
    res = bass_utils.run_bass_kernel_spmd(
        nc, in_maps, core_ids=list(range(cfg.N_CORES)))
    partials = [res.results[c]["partial"] for c in range(cfg.N_CORES)]
    out = epilogue(partials, cnt,
                   np.asarray(W2, np.float32), np.asarray(b2, np.float32),
                   np.asarray(fc1_W, np.float32), np.asarray(fc1_b, np.float32),
                   np.asarray(fc2_W, np.float32), np.asarray(fc2_b, np.float32))
    return out



# revision 31
# speedup vs baseline: 1.2394x; 1.1521x over previous
"""Trainium2 Bass kernel for a 2-layer GCN + global mean pool + MLP head.

Device work per core (SPMD, shared NEFF):
  - agg = A_norm @ x for the core's 12.5K-node shard: per-edge dma_gather of
    fp16 x rows (256B elems) + one-hot window matmuls into PSUM.
    * 50 gathers (25 blocks x 2 src chunks), round-robined over 4 SWDGE
      queues so descriptor generation uses all 8 Q7 cores.
    * src chunks of 50K rows via signed int16 indices against a centered
      base (the gather ucode sign-extends and MULUS-accumulates).
    * one-hot tiles precomputed on host (graph metadata only) and DMA'd,
      keeping the vector engine almost idle (it contends with SWDGE).
  - h1e = elu(agg.T @ W1 + b1); elementwise on the scalar engine.
  - partial[G, D] += wT_block.T @ h1e with wT = (B^T A_norm).T from host.
  - Host epilogue: sum partials, @W2, mean, fc1/relu/fc2, log_softmax.
"""
import sys
import numpy as np

sys.path.insert(0, "/opt/trn_rl_repo")


# ---------------------------------------------------------------- config ----
class CFG:
    def __init__(self, N=100000, D=128, G=64, n_cores=8, n_chunk=2, blk=512,
                 w=48, wseg=48):
        self.N, self.D, self.G = N, D, G
        self.N_CORES, self.N_CHUNK, self.BLK, self.W = n_cores, n_chunk, blk, w
        self.WSEG = wseg
        self.SHARD = N // n_cores
        self.CHUNK = N // n_chunk
        self.HALF = self.CHUNK // 2  # centered gather base offset
        assert self.HALF <= 32768
        self.N_BLK = (self.SHARD + blk - 1) // blk
        self.SHARD_PAD = self.N_BLK * blk
        self.N_RUN = self.N_BLK * n_chunk
        self.NT = blk // 128


FULL = CFG()


# ---------------------------------------------------------- preprocessing ----
def pack_run_joint(drels, W, blk):
    """Joint greedy window packing across cores: one shared base list; each
    core fills <=128 of its own (sorted) edges per tile."""
    nc_ = len(drels)
    ptrs = [0] * nc_
    tile_ofs = [np.full(len(d), -1, dtype=np.int64) for d in drels]
    wbases = []
    t = 0
    while True:
        lo = blk
        for c in range(nc_):
            if ptrs[c] < len(drels[c]):
                lo = min(lo, int(drels[c][ptrs[c]]))
        if lo >= blk:
            break
        base = min(lo, blk - W)
        for c in range(nc_):
            a = ptrs[c]
            if a >= len(drels[c]):
                continue
            hi = np.searchsorted(drels[c], base + W, side="left")
            j = min(a + 128, hi)
            tile_ofs[c][a:j] = t
            ptrs[c] = j
        wbases.append(base)
        t += 1
    return tile_ofs, wbases


def _sentinel_guard(gidx, oh, t_end):
    """The gather ucode drops trailing-negative indices; make sure the last
    slot of the (half-)gather ending at tile t_end is >= 0 by swapping a
    non-negative slot of the final tile into position."""
    last = t_end * 128 - 1
    if gidx[last] < 0:
        lt = t_end - 1
        cand = np.nonzero(gidx[lt * 128:(lt + 1) * 128] >= 0)[0]
        assert len(cand) > 0, "all-negative tile"
        sw = lt * 128 + int(cand[0])
        gidx[sw], gidx[last] = gidx[last], gidx[sw]
        tmp = oh[lt, cand[0]].copy()
        oh[lt, cand[0]] = oh[lt, 127]
        oh[lt, 127] = tmp


def preprocess(cfg, edge_index, batch):
    import ml_dtypes
    src = np.asarray(edge_index[0], dtype=np.int64)
    dst = np.asarray(edge_index[1], dtype=np.int64)
    batch = np.asarray(batch, dtype=np.int64)
    N, G, W, BLK = cfg.N, cfg.G, cfg.W, cfg.BLK

    deg = np.bincount(dst, minlength=N).astype(np.float32) + 1.0
    dinv = (1.0 / np.sqrt(deg)).astype(np.float32)

    loops = np.arange(N, dtype=np.int64)
    la = np.concatenate([src, loops])
    lb = np.concatenate([dst, loops])
    wnorm = (dinv[la] * dinv[lb]).astype(np.float32)
    flat = batch[lb] * N + la
    wmat = np.bincount(flat, weights=wnorm.astype(np.float64),
                       minlength=G * N).reshape(G, N).astype(np.float32)

    # self-loops are folded into the PSUM init (dinv^2-scaled x block rows
    # via DMA-transpose); only real edges go through the gather stream.
    # Duplicate (src,dst) pairs are merged with summed weights (aggregation
    # is linear), trimming ~8% of the per-edge gather stream.
    norm_full = (dinv[src] * dinv[dst]).astype(np.float32)
    key = dst * np.int64(N) + src
    uk, inv = np.unique(key, return_inverse=True)
    norm_all = np.bincount(inv, weights=norm_full.astype(np.float64)
                           ).astype(np.float32)
    src_all = (uk % N).astype(np.int64)
    dst_all = (uk // N).astype(np.int64)

    # Split each core's edges into two streams:
    #  - SEG: the first edge of each unique src, per block, window-sorted.
    #    These rows are laid out contiguously (slot order) in a per-core
    #    permuted copy of x, so the whole stream is bulk affine DMA with
    #    zero Q7 descriptor-generation work.
    #  - REP: all remaining edges, delivered via per-edge dma_gather.
    WSEG = cfg.WSEG
    cores = []
    cores_seg = []
    for c in range(cfg.N_CORES):
        m = (dst_all >= c * cfg.SHARD) & (dst_all < (c + 1) * cfg.SHARD)
        s_a = src_all[m]
        dl_a = dst_all[m] - c * cfg.SHARD
        nv_a = norm_all[m]
        blk_a = dl_a // BLK
        wrel_a = dl_a - blk_a * BLK
        o1 = np.lexsort((wrel_a, blk_a))
        s_a, dl_a, nv_a = s_a[o1], dl_a[o1], nv_a[o1]
        blk_a, wrel_a = blk_a[o1], wrel_a[o1]
        _, first_pos = np.unique(s_a, return_index=True)
        isf = np.zeros(len(s_a), dtype=bool)
        isf[first_pos] = True
        # seg stream, already sorted by (block, wrel)
        sb = blk_a[isf]
        seg_starts = np.searchsorted(sb, np.arange(cfg.N_BLK))
        seg_ends = np.searchsorted(sb, np.arange(cfg.N_BLK) + 1)
        cores_seg.append((s_a[isf], wrel_a[isf], nv_a[isf],
                          seg_starts, seg_ends))
        # repeat stream through the existing run machinery
        s_c, dl_c, nv_c = s_a[~isf], dl_a[~isf], nv_a[~isf]
        blk_id = dl_c // BLK
        chunk = s_c // cfg.CHUNK
        o = np.lexsort((dl_c, chunk, blk_id))
        s_c, dl_c, nv_c = s_c[o], dl_c[o], nv_c[o]
        run_id = blk_id[o] * cfg.N_CHUNK + chunk[o]
        run_starts = np.searchsorted(run_id, np.arange(cfg.N_RUN))
        run_ends = np.searchsorted(run_id, np.arange(cfg.N_RUN) + 1)
        cores.append((s_c, dl_c, nv_c, run_starts, run_ends))

    # SEG joint packing per block (shared tile/window structure: one NEFF)
    seg_T, seg_wb = [], []
    core_segsrc = [[] for _ in range(cfg.N_CORES)]
    core_segoh = [[] for _ in range(cfg.N_CORES)]
    for b in range(cfg.N_BLK):
        drels = []
        for c in range(cfg.N_CORES):
            _, wr, _, ss, se = cores_seg[c]
            drels.append(wr[int(ss[b]):int(se[b])])
        tile_ofs, wbases = pack_run_joint(drels, WSEG, BLK)
        T_b = max(len(wbases), 1)
        wbases = wbases or [0]
        wb_arr = np.asarray(wbases, dtype=np.int64)
        for c in range(cfg.N_CORES):
            sr, wr, nv, ss, se = cores_seg[c]
            a, e = int(ss[b]), int(se[b])
            tile_of = tile_ofs[c]
            srcmap = np.full(T_b * 128, -1, dtype=np.int64)
            oh = np.zeros((T_b, 128, WSEG), dtype=ml_dtypes.float8_e4m3)
            if e > a:
                loads = np.bincount(tile_of, minlength=T_b)
                cum = np.concatenate([[0], np.cumsum(loads)])[:-1]
                pos = np.arange(e - a) - np.repeat(cum, loads)
                slot = tile_of * 128 + pos
                srcmap[slot] = sr[a:e]
                wrel = (wr[a:e] - wb_arr[tile_of]).astype(np.int64)
                oh[tile_of, pos, wrel] = nv[a:e].astype(
                    ml_dtypes.float8_e4m3)
            core_segsrc[c].append(srcmap)
            core_segoh[c].append(oh.transpose(1, 0, 2).reshape(128,
                                                               T_b * WSEG))
        seg_T.append(T_b)
        seg_wb.append(wbases)

    run_T, run_wb = [], []
    core_gidx = [[] for _ in range(cfg.N_CORES)]
    core_oh = [[] for _ in range(cfg.N_CORES)]
    for r in range(cfg.N_RUN):
        bb = (r // cfg.N_CHUNK) * BLK
        ch = r % cfg.N_CHUNK
        drels = []
        for c in range(cfg.N_CORES):
            s_c, dl_c, nv_c, rs, re = cores[c]
            a, b = int(rs[r]), int(re[r])
            drels.append(dl_c[a:b] - bb)
        tile_ofs, wbases = pack_run_joint(drels, W, BLK)
        T_r = max(len(wbases), 1)
        wbases = wbases or [0]
        S_r = T_r * 128
        wb_arr = np.asarray(wbases, dtype=np.int64)
        for c in range(cfg.N_CORES):
            s_c, dl_c, nv_c, rs, re = cores[c]
            a, b = int(rs[r]), int(re[r])
            tile_of = tile_ofs[c]
            assert b == a or (tile_of >= 0).all()
            gidx = np.zeros(S_r, dtype=np.int16)
            oh = np.zeros((T_r, 128, W), dtype=ml_dtypes.float8_e4m3)
            if b > a:
                loads = np.bincount(tile_of, minlength=T_r)
                cum = np.concatenate([[0], np.cumsum(loads)])[:-1]
                pos = np.arange(b - a) - np.repeat(cum, loads)
                slot = tile_of * 128 + pos
                gidx[slot] = (s_c[a:b] - ch * cfg.CHUNK
                              - cfg.HALF).astype(np.int16)
                wrel = (dl_c[a:b] - bb - wb_arr[tile_of]).astype(np.int64)
                oh[tile_of, pos, wrel] = nv_c[a:b].astype(
                    ml_dtypes.float8_e4m3)
                # the gather is issued as two halves; guard both boundaries
                t_half = (T_r + 1) // 2
                _sentinel_guard(gidx, oh, t_half)
                if T_r > t_half:
                    _sentinel_guard(gidx, oh, T_r)
            gw = np.tile(gidx.reshape(S_r // 16, 16).T, (8, 1))
            core_gidx[c].append(gw)
            core_oh[c].append(oh.transpose(1, 0, 2).reshape(128, T_r * W))
        run_T.append(T_r)
        run_wb.append(wbases)

    per_core = []
    for c in range(cfg.N_CORES):
        wT = np.zeros((cfg.SHARD_PAD, G), dtype=ml_dtypes.float8_e4m3)
        wT[:cfg.SHARD] = wmat[:, c * cfg.SHARD:(c + 1) * cfg.SHARD].T
        per_core.append({
            "gidx": np.ascontiguousarray(np.concatenate(core_gidx[c], axis=1)),
            "oh": np.ascontiguousarray(np.concatenate(core_oh[c], axis=1)),
            "wT": wT,
            "dinv2": (dinv[c * cfg.SHARD:(c + 1) * cfg.SHARD] ** 2
                      ).astype(np.float32),
            "segsrc": np.concatenate(core_segsrc[c]),
            "ohseg": np.ascontiguousarray(
                np.concatenate(core_segoh[c], axis=1)),
            # shared NEFF metadata (identical for every core)
            "seg_T": seg_T,
            "seg_wb": seg_wb,
        })

    cnt = np.bincount(batch, minlength=G).astype(np.float32)
    return per_core, cnt, run_T, run_wb


# ---------------------------------------------------------- bass kernel ----
def build_kernel(cfg, run_T, run_wb, seg_T, seg_wb):
    from concourse import bacc, bass, tile, mybir
    from concourse.masks import make_identity
    f32 = mybir.dt.float32
    f16 = mybir.dt.float16
    bf16 = mybir.dt.bfloat16
    f8e4 = mybir.dt.float8e4
    f8e3 = mybir.dt.float8e3

    T_total = sum(run_T)
    Tseg_total = sum(seg_T)
    WSEG = cfg.WSEG
    nc = bacc.Bacc("TRN2", target_bir_lowering=False, debug=False,
                   enable_asserts=False, num_swdge_queues=4,
                   dynamic_dma_scratch_size=40960)
    x_t = nc.dram_tensor("x", [cfg.N, cfg.D], f16, kind="ExternalInput")
    xself_t = nc.dram_tensor("x_selfT", [cfg.D, cfg.SHARD_PAD], f8e3,
                             kind="ExternalInput")
    gidx_t = nc.dram_tensor("gidx", [128, 8 * T_total], mybir.dt.int16,
                            kind="ExternalInput")
    oh_t = nc.dram_tensor("oh", [128, cfg.W * T_total], f8e4,
                          kind="ExternalInput")
    xperm_t = nc.dram_tensor("xperm", [128, cfg.D * Tseg_total], f8e3,
                             kind="ExternalInput")
    ohseg_t = nc.dram_tensor("ohseg", [128, WSEG * Tseg_total], f8e4,
                             kind="ExternalInput")
    wT_t = nc.dram_tensor("wT", [cfg.SHARD_PAD, cfg.G], f8e4,
                          kind="ExternalInput")
    W1_t = nc.dram_tensor("W1", [cfg.D, cfg.D], bf16, kind="ExternalInput")
    b1_t = nc.dram_tensor("b1", [1, cfg.D], f32, kind="ExternalInput")
    out_t = nc.dram_tensor("partial", [cfg.G, cfg.D], f32,
                           kind="ExternalOutput")

    W, NT, BLK = cfg.W, cfg.NT, cfg.BLK
    add, amax = mybir.AluOpType.add, mybir.AluOpType.max

    with tile.TileContext(nc) as tc:
        with (tc.tile_pool(name="const", bufs=1) as cpool,
              tc.tile_pool(name="gbuf", bufs=6) as gpool,
              tc.tile_pool(name="meta", bufs=6) as mpool,
              tc.tile_pool(name="ohp", bufs=6) as opool,
              tc.tile_pool(name="segx", bufs=2) as spool,
              tc.tile_pool(name="sohp", bufs=2) as sopool,
              tc.tile_pool(name="agg", bufs=2) as apool,
              tc.tile_pool(name="xbtp", bufs=4) as xpool,
              tc.tile_pool(name="eluv", bufs=2) as epool,
              tc.tile_pool(name="wt", bufs=4) as wpool,
              tc.tile_pool(name="psA", bufs=2, space="PSUM") as pApool,
              tc.tile_pool(name="psB", bufs=2, space="PSUM") as pBpool,
              tc.tile_pool(name="psR", bufs=2, space="PSUM") as pRpool,
              tc.tile_pool(name="psC", bufs=1, space="PSUM") as pCpool,
              tc.tile_pool(name="outp", bufs=1) as outpool):

            # rolling metadata prefetch PF runs ahead: every load in the
            # (in-order) Sync HWDGE stream is issued well before its consumer
            # so no wait in that stream ever blocks later loads -- otherwise
            # the whole pipeline marches in lockstep at one block per drain.
            PF = 4
            run_off = np.concatenate([[0], np.cumsum(run_T)]).astype(int)

            def load_meta(rr):
                off = int(run_off[rr])
                T_rr = run_T[rr]
                gxs = mpool.tile([128, 8 * T_rr], mybir.dt.int16)
                nc.sync.dma_start(
                    gxs[:], gidx_t.ap()[:, 8 * off:8 * (off + T_rr)])
                ohs = opool.tile([128, T_rr, W], f8e4)
                nc.sync.dma_start(
                    ohs[:].rearrange("p t w -> p (t w)"),
                    oh_t.ap()[:, W * off:W * (off + T_rr)])
                return gxs, ohs

            pre = {}
            for rr in range(min(PF, cfg.N_RUN)):
                pre[rr] = load_meta(rr)

            ident = cpool.tile([128, 128], f16)
            make_identity(nc, ident[:])
            W1s = cpool.tile([128, cfg.D], bf16)
            nc.sync.dma_start(W1s[:], W1_t.ap())
            b1s = cpool.tile([128, cfg.D], f32)
            nc.sync.dma_start(b1s[:], b1_t.ap().to_broadcast((128, cfg.D)))

            psC = pCpool.tile([cfg.G, cfg.D], f32)

            seg_off = np.concatenate([[0], np.cumsum(seg_T)]).astype(int)
            for b in range(cfg.N_BLK):
                # per-block side streams, hoisted to the block top so their
                # (slack, 4-deep) pool waits sit early in the Sync stream
                xbt = xpool.tile([128, BLK], f8e3)
                nc.sync.dma_start(
                    xbt[:], xself_t.ap()[:, b * BLK:(b + 1) * BLK])
                wts = wpool.tile([128, NT, cfg.G], f8e4)
                nc.sync.dma_start(
                    wts[:],
                    wT_t.ap()[b * BLK:(b + 1) * BLK, :]
                        .rearrange("(t p) g -> p t g", p=128))

                # SEG stream: first-appearance rows, pre-permuted on the host
                # into slot order -- one big affine DMA, no Q7 descriptors.
                T_b = seg_T[b]
                so = int(seg_off[b])
                segx = spool.tile([128, T_b, cfg.D], f8e3)
                nc.sync.dma_start(
                    segx[:].rearrange("p t d -> p (t d)"),
                    xperm_t.ap()[:, cfg.D * so:cfg.D * (so + T_b)])
                sohs = sopool.tile([128, T_b, WSEG], f8e4)
                nc.sync.dma_start(
                    sohs[:].rearrange("p t w -> p (t w)"),
                    ohseg_t.ap()[:, WSEG * so:WSEG * (so + T_b)])

                # init psA with the self-loop term: dinv^2-scaled x rows of
                # this block, host-pretransposed to feat-major
                psA = pApool.tile([128, BLK], f32)
                nc.tensor.matmul(out=psA[:], lhsT=ident[:],
                                 rhs=xbt[:], start=True, stop=False)
                swbs = seg_wb[b]
                for t in range(T_b):
                    nc.tensor.matmul(
                        out=psA[:, swbs[t]:swbs[t] + WSEG],
                        lhsT=segx[:, t, :],
                        rhs=sohs[:, t, :],
                        start=False, stop=False,
                    )

                for ch in range(cfg.N_CHUNK):
                    r = b * cfg.N_CHUNK + ch
                    T_r = run_T[r]
                    gxs, ohs = pre.pop(r)
                    if r + PF < cfg.N_RUN:
                        pre[r + PF] = load_meta(r + PF)

                    gb = gpool.tile([128, T_r, cfg.D], f16)
                    base_row = ch * cfg.CHUNK + cfg.HALF
                    # one gather per run: the DMASW sem rotation (8 lanes)
                    # caps in-flight SWDGE DMAs at 8, so fewer/bigger gathers
                    # maximize the descriptor runway the DMA engines can
                    # drain ahead (rings hold ~2 full gathers per queue).
                    S_r = T_r * 128
                    nc.gpsimd.dma_gather(
                        out_ap=gb[:],
                        in_ap=x_t.ap()[base_row:cfg.N, :],
                        idxs_ap=gxs[:],
                        num_idxs=S_r,
                        num_idxs_reg=S_r,
                        elem_size=cfg.D,
                        single_packet=False,
                        queue_num=r % 4,
                    )

                    wbs = run_wb[r]
                    for t in range(T_r):
                        last = (ch == cfg.N_CHUNK - 1 and t == T_r - 1)
                        nc.tensor.matmul(
                            out=psA[:, wbs[t]:wbs[t] + W],
                            lhsT=gb[:, t, :],
                            rhs=ohs[:, t, :],
                            start=False, stop=last,
                        )

                # drain agg (feat-major [D x BLK])
                aggs = apool.tile([128, BLK], bf16)
                nc.vector.tensor_copy(out=aggs[:], in_=psA[:])

                # B: h1 = agg.T @ W1  -> psB [node x feat_out]
                psB = pBpool.tile([128, BLK], f32)
                for nt in range(NT):
                    nc.tensor.matmul(out=psB[:, nt * cfg.D:(nt + 1) * cfg.D],
                                     lhsT=aggs[:, nt * 128:(nt + 1) * 128],
                                     rhs=W1s[:], start=True, stop=True)

                # elu(xb) = relu(xb) - relu(1 - exp(xb)), xb = psB + b1.
                # Every DVE op reads at most ONE SBUF operand (the other is
                # PSUM) -- 2-SBUF-port DVE ops get locked out by concurrent
                # SWDGE descriptor generation (measured 100x slowdown).
                xb = epool.tile([128, NT, cfg.D], f32, tag="xb")
                nc.vector.tensor_tensor(
                    out=xb[:],
                    in0=psB[:].rearrange("p (t d) -> p t d", d=cfg.D),
                    in1=b1s[:].unsqueeze(1).broadcast_to((128, NT, cfg.D)),
                    op=add)
                ex = epool.tile([128, NT * cfg.D], f32, tag="ex")
                nc.scalar.activation(
                    out=ex[:], in_=xb[:].rearrange("p t d -> p (t d)"),
                    func=mybir.ActivationFunctionType.Exp)
                rneg = epool.tile([128, NT * cfg.D], f32, tag="rneg")
                nc.scalar.activation(
                    out=rneg[:], in_=ex[:],
                    func=mybir.ActivationFunctionType.Relu,
                    bias=1.0, scale=-1.0)
                rpos = pRpool.tile([128, NT * cfg.D], f32)
                nc.scalar.activation(
                    out=rpos[:], in_=xb[:].rearrange("p t d -> p (t d)"),
                    func=mybir.ActivationFunctionType.Relu)
                h1e = epool.tile([128, NT * cfg.D], bf16, tag="h1e")
                nc.vector.tensor_tensor(
                    out=h1e[:], in0=rpos[:], in1=rneg[:],
                    op=mybir.AluOpType.subtract)

                # C: partial += wT_block.T @ h1e
                for nt in range(NT):
                    nc.tensor.matmul(
                        out=psC[:],
                        lhsT=wts[:, nt, :],
                        rhs=h1e[:, nt * cfg.D:(nt + 1) * cfg.D],
                        start=(b == 0 and nt == 0),
                        stop=(b == cfg.N_BLK - 1 and nt == NT - 1),
                    )

            outs = outpool.tile([cfg.G, cfg.D], f32)
            nc.vector.tensor_copy(out=outs[:], in_=psC[:])
            nc.sync.dma_start(out_t.ap(), outs[:])

    nc.compile()
    return nc


# ------------------------------------------------------------- epilogue ----
def epilogue(partials, cnt, W2, b2, fc1_W, fc1_b, fc2_W, fc2_b):
    g_sum = np.sum(partials, axis=0, dtype=np.float32)
    S = g_sum @ W2 + cnt[:, None] * b2[None, :]
    mean = S / np.maximum(cnt, 1.0)[:, None]
    z = np.maximum(mean @ fc1_W + fc1_b[None, :], 0.0)
    z = z @ fc2_W + fc2_b[None, :]
    zmax = z.max(axis=1, keepdims=True)
    lse = np.log(np.sum(np.exp(z - zmax), axis=1, keepdims=True)) + zmax
    return (z - lse).astype(np.float32)


_NC_CACHE = {}


def run_on_device(cfg, per_core, run_T, run_wb, x):
    seg_T = per_core[0]["seg_T"]
    seg_wb = per_core[0]["seg_wb"]
    key = (tuple(run_T), tuple(tuple(w) for w in run_wb),
           tuple(seg_T), tuple(tuple(w) for w in seg_wb))
    if key not in _NC_CACHE:
        _NC_CACHE.clear()
        _NC_CACHE[key] = build_kernel(cfg, run_T, run_wb, seg_T, seg_wb)
    nc = _NC_CACHE[key]
    import ml_dtypes
    xf = np.asarray(x, np.float32)
    xp = np.ascontiguousarray(xf.astype(np.float16))
    Tseg_total = sum(seg_T)
    in_maps = []
    for c in range(cfg.N_CORES):
        s = per_core[c]
        xs = np.zeros((cfg.SHARD_PAD, cfg.D), dtype=ml_dtypes.float8_e3m4)
        xs[:cfg.SHARD] = (xf[c * cfg.SHARD:(c + 1) * cfg.SHARD]
                          * s["dinv2"][:, None]).astype(ml_dtypes.float8_e3m4)
        # per-core node-relabeled copy of x: seg slot order, SBUF layout
        segsrc = s["segsrc"]
        rows = np.zeros((Tseg_total * 128, cfg.D),
                        dtype=ml_dtypes.float8_e3m4)
        valid = segsrc >= 0
        rows[valid] = xf[segsrc[valid]].astype(ml_dtypes.float8_e3m4)
        xperm = np.ascontiguousarray(
            rows.reshape(Tseg_total, 128, cfg.D)
                .transpose(1, 0, 2).reshape(128, Tseg_total * cfg.D))
        in_maps.append({
            "x": xp, "x_selfT": np.ascontiguousarray(xs.T),
            "gidx": s["gidx"], "oh": s["oh"],
            "xperm": xperm, "ohseg": s["ohseg"],
            "wT": s["wT"], "W1": None, "b1": None,
        })
    return nc, in_maps


def kernel(x, edge_index, batch, W1, b1, W2, b2, fc1_W, fc1_b, fc2_W, fc2_b):
    from concourse import bass_utils
    cfg = FULL
    per_core, cnt, run_T, run_wb = preprocess(cfg, edge_index, batch)
    nc, in_maps = run_on_device(cfg, per_core, run_T, run_wb, x)
    import ml_dtypes
    W1f = np.ascontiguousarray(
        np.asarray(W1, dtype=np.float32).astype(ml_dtypes.bfloat16))
    b1f = np.asarray(b1, dtype=np.float32).reshape(1, cfg.D)
    for m in in_maps:
        m["W1"] = W1f
        m["b1"] = b1f
    res = bass_utils.run_bass_kernel_spmd(
        nc, in_maps, core_ids=list(range(cfg.N_CORES)))
    partials = [res.results[c]["partial"] for c in range(cfg.N_CORES)]
    out = epilogue(partials, cnt,
                   np.asarray(W2, np.float32), np.asarray(b2, np.float32),
                   np.asarray(fc1_W, np.float32), np.asarray(fc1_b, np.float32),
                   np.asarray(fc2_W, np.float32), np.asarray(fc2_b, np.float32))
    return out

